# revision 14
# baseline (speedup 1.0000x reference)
"""Multi-head attention (RoPE, causal) Trainium2 Bass kernel, 8 NeuronCores.

Problem: x[4,2048,1024] -> MHA(16 heads, head_dim 64, RoPE, causal mask) -> [4,2048,1024]

Sharding (pure data/tensor parallel, no collectives):
  core c -> (batch b = c//2, head-group g = c%2); each head-group = 8 heads = 512 dims.
  Each core computes q/k/v projections for its (batch, head-group), RoPE, attention,
  and a partial output projection (columns of Wo for its head group).
  Host sums the two partial outputs per batch (512-dim contraction split).

Kernel layout tricks:
  - Projections computed in transposed [out_dim, seq] layout (QT/KT) so that
    QK^T blocks come out as S^T [k, q] with softmax along the free (q) dim never
    needed: we use UNSAFE softmax (no row-max; inputs are bounded N(0,1)-ish data,
    logits stay << 88) and fold the row-sum into the PV matmul by augmenting V
    with a ones column.  No on-chip transposes anywhere.
  - V is computed in natural [seq, dim] layout (lhsT = xT chunks) for PV.
  - RoPE via a signed-permutation matrix on the TensorEngine (rot = Psig @ QT)
    plus 3 VectorEngine elementwise ops per chunk.
  - All matmuls in float32r (full-rate fp32 on TRN2 for moving dim >= 256).
  - Causal masking: lower blocks computed unmasked, diagonal-strip blocks get a
    0/1 mask multiply; upper blocks skipped entirely.
"""

import numpy as np

import concourse.bass as bass
import concourse.tile as tile
from concourse import bacc, mybir
from concourse import bass_utils

B, S, D, H, DH = 4, 2048, 1024, 16, 64
NCORES = 8
HG = 2              # head groups (tensor parallel)
HPG = H // HG       # heads per group = 8
OG = HPG * DH       # group output dims = 512
SCALE = DH ** -0.5
P = 128
QSB = 512           # q super-block width
NQSB = S // QSB     # 4
KB = 128            # k block
NKB = S // KB       # 16
DC = D // P         # 8 d-chunks
JC = OG // P        # 4 j-chunks (out-proj contraction)

F32 = mybir.dt.float32
F32R = mybir.dt.float32r
BF16 = mybir.dt.bfloat16

_COMPILED = {}      # plan_key -> (nc, uses_dram_mask)


# ---------------------------------------------------------------- host tables

def _rope_tables():
    inv_freq = 1.0 / (10000.0 ** (np.arange(0, DH, 2, dtype=np.float32) / DH))
    t = np.arange(S, dtype=np.float32)
    freqs = np.outer(t, inv_freq).astype(np.float32)      # [S, 32]
    emb = np.concatenate([freqs, freqs], -1)              # [S, 64]
    return np.cos(emb), np.sin(emb)


def _host_consts():
    cos, sin = _rope_tables()                             # [S, 64]
    cosT2 = np.ascontiguousarray(
        np.concatenate([cos.T, cos.T], axis=0), dtype=np.float32)   # [128, S]
    sinT2 = np.ascontiguousarray(
        np.concatenate([sin.T, sin.T], axis=0), dtype=np.float32)
    # signed permutation: rot(x)[i] = -x[i+32] (j<32) else x[i-32], per 64-row head
    psig = np.zeros((P, P), np.float32)
    for i in range(P):
        j = i % DH
        base = (i // DH) * DH
        if j < 32:
            psig[i, base + j + 32] = -1.0
        else:
            psig[i, base + j - 32] = 1.0
    psigT = np.ascontiguousarray(psig.T)
    return cosT2, sinT2, psigT


def _mask_plan(mask):
    """Classify the [S, S] mask into a per-qsb block plan.

    Returns (plan, mode) where plan[qsb] is a list of (kb, msel) and msel is
    None (no mask), ("const", r) for the 4 shared causal diagonal tiles, or
    ("dram", qsb, kb) for generic per-block mask tiles.
    """
    m = np.asarray(mask).reshape(S, S) != 0        # [q, k] True = attend
    causal = np.array_equal(m, np.tril(np.ones((S, S), bool)))
    if causal:
        plan = []
        for qsb in range(NQSB):
            row = []
            for kb in range(4 * qsb + 4):
                r = kb - 4 * qsb
                row.append((kb, None if r < 0 else ("const", r)))
            plan.append(row)
        return plan, "causal"
    if m.all():
        return [[(kb, None) for kb in range(NKB)] for _ in range(NQSB)], "full"
    plan = []
    for qsb in range(NQSB):
        row = []
        for kb in range(NKB):
            blk = m[qsb * QSB:(qsb + 1) * QSB, kb * KB:(kb + 1) * KB]  # [q, k]
            if not blk.any():
                continue          # fully masked block contributes nothing
            row.append((kb, None if blk.all() else ("dram", qsb, kb)))
        plan.append(row)
    return plan, "generic"


# ------------------------------------------------------------------- builder

def _build(plan, mode):
    nc = bacc.Bacc("TRN2", target_bir_lowering=False, debug=False, num_devices=1)
    AF = mybir.ActivationFunctionType
    OP = mybir.AluOpType

    xT_d = nc.dram_tensor("xT", [D, S], F32R, kind="ExternalInput").ap()
    wqT_d = nc.dram_tensor("wqT", [D, OG], F32R, kind="ExternalInput").ap()
    wkT_d = nc.dram_tensor("wkT", [D, OG], F32R, kind="ExternalInput").ap()
    wvT_d = nc.dram_tensor("wvT", [D, OG], F32R, kind="ExternalInput").ap()
    woT_d = nc.dram_tensor("woT", [OG, D], F32R, kind="ExternalInput").ap()
    cos_d = nc.dram_tensor("cosT", [P, S], F32, kind="ExternalInput").ap()
    sin_d = nc.dram_tensor("sinT", [P, S], F32, kind="ExternalInput").ap()
    psg_d = nc.dram_tensor("psgT", [P, P], F32R, kind="ExternalInput").ap()
    if mode == "causal":
        m01_d = nc.dram_tensor("m01", [4, P, QSB], F32, kind="ExternalInput").ap()
    elif mode == "generic":
        m01_d = nc.dram_tensor("m01", [NQSB, NKB, P, QSB], F32,
                               kind="ExternalInput").ap()
    else:
        m01_d = None
    one64_d = nc.dram_tensor("one64", [1, DH], F32R, kind="ExternalInput").ap()
    outT_d = nc.dram_tensor("outT", [D, S], F32, kind="ExternalOutput").ap()

    with tile.TileContext(nc) as tc:
        from contextlib import ExitStack
        with ExitStack() as ctx:
            persist = ctx.enter_context(tc.tile_pool(name="persist", bufs=1))
            wstream = ctx.enter_context(tc.tile_pool(name="wstream", bufs=2))
            work = ctx.enter_context(tc.tile_pool(name="work", bufs=2))
            ptpool = ctx.enter_context(tc.tile_pool(name="ptpool", bufs=3))
            ppool = ctx.enter_context(
                tc.tile_pool(name="ppool", bufs=4, space="PSUM"))
            pvp = ctx.enter_context(
                tc.tile_pool(name="pvp", bufs=3, space="PSUM"))

            # ---------------- phase 1: projections + rope ----------------
            ph1 = tc.tile_pool(name="ph1", bufs=1)
            p1 = ph1.__enter__()

            xT = p1.tile([P, DC, S], F32R)
            for sc in range(4):
                nc.sync.dma_start(
                    xT[:, :, sc * QSB:(sc + 1) * QSB],
                    xT_d.rearrange("(c p) s -> p c s", p=P)
                        [:, :, sc * QSB:(sc + 1) * QSB])
            cos_sb = p1.tile([P, S], F32)
            sin_sb = p1.tile([P, S], F32)
            nc.sync.dma_start(cos_sb[:], cos_d)
            nc.sync.dma_start(sin_sb[:], sin_d)
            psg_sb = p1.tile([P, P], F32R)
            nc.sync.dma_start(psg_sb[:], psg_d)

            QT = [persist.tile([P, S], F32R, tag=f"qt{t}", name=f"qt{t}") for t in range(4)]
            KT = [persist.tile([P, S], F32R, tag=f"kt{t}", name=f"kt{t}") for t in range(4)]
            # V with ones column: [p, s-block, head, 65]
            V = persist.tile([P, NKB, HPG, DH + 1], BF16, tag="v")
            nc.vector.memset(V[:, :, :, DH:DH + 1], 1.0)

            # Q/K projections -> transposed [o, s] layout
            for w_d, dst in ((wqT_d, QT), (wkT_d, KT)):
                for oc in range(4):
                    w_oc = wstream.tile([P, DC, P], F32R, tag="wqk")
                    nc.sync.dma_start(
                        w_oc[:],
                        w_d.rearrange("(c p) o -> p c o", p=P)
                           [:, :, oc * P:(oc + 1) * P])
                    for sc in range(4):
                        ps = ppool.tile([P, QSB], F32, tag="ps", name="ps")
                        for dc in range(DC):
                            nc.tensor.matmul(
                                ps[:],
                                w_oc[:, dc, :],
                                xT[:, dc, sc * QSB:(sc + 1) * QSB],
                                start=(dc == 0), stop=(dc == DC - 1))
                        nc.scalar.copy(dst[oc][:, sc * QSB:(sc + 1) * QSB], ps[:])

            # V projection -> natural [s, o] layout
            wv = p1.tile([P, DC, OG], F32R)
            nc.sync.dma_start(wv[:], wvT_d.rearrange("(c p) o -> p c o", p=P))
            for sb in range(NKB):
                ps = ppool.tile([P, QSB], F32, tag="ps", name="ps")
                for dc in range(DC):
                    nc.tensor.matmul(
                        ps[:],
                        xT[:, dc, sb * P:(sb + 1) * P],
                        wv[:, dc, :],
                        start=(dc == 0), stop=(dc == DC - 1))
                nc.scalar.copy(
                    V[:, sb, :, 0:DH],
                    ps[:].rearrange("p (h j) -> p h j", j=DH))

            # RoPE in place on QT/KT tiles
            for tiles in (QT, KT):
                for t in range(4):
                    for c in range(4):
                        sl = slice(c * QSB, (c + 1) * QSB)
                        rot = ppool.tile([P, QSB], F32, tag="ps", name="rot")
                        nc.tensor.matmul(
                            rot[:], psg_sb[:],
                            tiles[t][:, sl],
                            start=True, stop=True)
                        m = work.tile([P, QSB], F32, tag="ropem")
                        nc.vector.tensor_tensor(
                            m[:], tiles[t][:, sl], cos_sb[:, sl], OP.mult)
                        r = work.tile([P, QSB], F32, tag="roper")
                        nc.vector.tensor_tensor(
                            r[:], rot[:], sin_sb[:, sl], OP.mult)
                        nc.vector.tensor_tensor(
                            tiles[t][:, sl], m[:], r[:], OP.add)

            ph1.__exit__(None, None, None)

            # ---------------- phase 2: attention ----------------
            ph2 = tc.tile_pool(name="ph2", bufs=1)
            p2 = ph2.__enter__()

            aT = [p2.tile([P, S], F32R, tag=f"at{t}", name=f"at{t}") for t in range(4)]
            ones64 = p2.tile([1, DH], F32R, tag="ones64")
            nc.sync.dma_start(ones64[:], one64_d)
            if mode == "causal":
                mk = p2.tile([P, 4, QSB], BF16, tag="m01")
                for r in range(4):
                    mkf = work.tile([P, QSB], F32, tag="ropem", name="mkf")
                    nc.sync.dma_start(mkf[:], m01_d[r])
                    nc.vector.tensor_copy(mk[:, r, :], mkf[:])

            for h in range(HPG):
                tq = h // 2
                ph = (h % 2) * DH
                for qsb in range(NQSB):
                    qsl = slice(qsb * QSB, (qsb + 1) * QSB)
                    q_ap = QT[tq][ph:ph + DH, qsl]
                    blocks = plan[qsb]
                    pv = pvp.tile([DH + 1, QSB], F32, tag="pv", name="pv")
                    for i, (kb, msel) in enumerate(blocks):
                        st = ppool.tile([P, QSB], F32, tag="ps", name="st")
                        nc.tensor.matmul(
                            st[:],
                            KT[tq][ph:ph + DH, kb * KB:(kb + 1) * KB],
                            q_ap,
                            start=True, stop=True)
                        pt = ptpool.tile([P, QSB], BF16, tag="pt")
                        nc.scalar.activation(pt[:], st[:], AF.Exp, scale=SCALE)
                        if msel is not None:
                            if msel[0] == "const":
                                nc.vector.tensor_tensor(
                                    pt[:], pt[:], mk[:, msel[1], :], OP.mult)
                            else:
                                mg = work.tile([P, QSB], F32, tag="ropem")
                                nc.sync.dma_start(
                                    mg[:], m01_d[msel[1], msel[2]])
                                mgb = ptpool.tile([P, QSB], BF16, tag="pt")
                                nc.vector.tensor_copy(mgb[:], mg[:])
                                nc.vector.tensor_tensor(
                                    pt[:], pt[:], mgb[:], OP.mult)
                        nc.tensor.matmul(
                            pv[:],
                            V[:, kb, h, :],
                            pt[:],
                            start=(i == 0), stop=(i == len(blocks) - 1))
                    # normalize directly from the PV accumulator:
                    # 1/l -> PE-broadcast over the head's 64 rows -> fused into aT
                    linv = work.tile([1, QSB], F32R, tag="linv")
                    with nc.allow_low_precision(reason="f32r rounding of 1/l"):
                        nc.vector.reciprocal(linv[:], pv[DH:DH + 1, :])
                    bc = ppool.tile([P, QSB], F32, tag="ps", name="bc")
                    nc.tensor.matmul(
                        bc[0:DH, :], ones64[:],
                        linv[:], start=True, stop=True)
                    nc.vector.tensor_copy(aT[tq][ph:ph + DH, qsl], pv[0:DH, :])
                    nc.vector.tensor_tensor(
                        aT[tq][ph:ph + DH, qsl], aT[tq][ph:ph + DH, qsl],
                        bc[0:DH, :], OP.mult)

            # output projection (partial over this head group's 512 dims)
            for oc in range(8):
                wo = wstream.tile([P, JC, P], F32R, tag="wo")
                nc.sync.dma_start(
                    wo[:],
                    woT_d.rearrange("(c p) o -> p c o", p=P)
                         [:, :, oc * P:(oc + 1) * P])
                for sc in range(4):
                    ssl = slice(sc * QSB, (sc + 1) * QSB)
                    ps = ppool.tile([P, QSB], F32, tag="ps", name="ps")
                    for jc in range(JC):
                        nc.tensor.matmul(
                            ps[:], wo[:, jc, :],
                            aT[jc][:, ssl],
                            start=(jc == 0), stop=(jc == JC - 1))
                    stg = work.tile([P, QSB], F32, tag="ropem")
                    nc.scalar.copy(stg[:], ps[:])
                    nc.sync.dma_start(outT_d[oc * P:(oc + 1) * P, ssl], stg[:])

            ph2.__exit__(None, None, None)

    nc.compile()
    return nc


def _plan_key(plan, mode):
    return (mode, tuple(tuple(row) for row in plan))


def _get_compiled(mask):
    plan, mode = _mask_plan(mask)
    key = _plan_key(plan, mode)
    if key not in _COMPILED:
        _COMPILED[key] = (_build(plan, mode), plan, mode)
    return _COMPILED[key]


# --------------------------------------------------------------- host driver

def _make_in_maps(x, Wq, Wk, Wv, Wo, mask, mode):
    cosT2, sinT2, psigT = _host_consts()
    consts = {"cosT": cosT2, "sinT": sinT2, "psgT": psigT,
              "one64": np.ones((1, DH), np.float32)}
    if mode == "causal":
        m01 = np.zeros((4, P, QSB), np.float32)
        for r in range(4):
            for k in range(P):
                q0 = KB * r + k
                if q0 < QSB:
                    m01[r, k, q0:] = 1.0
        consts["m01"] = m01
    elif mode == "generic":
        m = (np.asarray(mask).reshape(S, S) != 0)
        m01 = np.zeros((NQSB, NKB, P, QSB), np.float32)
        for qsb in range(NQSB):
            for kb in range(NKB):
                blk = m[qsb * QSB:(qsb + 1) * QSB, kb * KB:(kb + 1) * KB]
                m01[qsb, kb] = blk.T.astype(np.float32)
        consts["m01"] = m01

    in_maps = []
    for c in range(NCORES):
        b, g = c // HG, c % HG
        rows = slice(OG * g, OG * (g + 1))
        in_maps.append({
            "xT": np.ascontiguousarray(x[b].T, dtype=np.float32),
            "wqT": np.ascontiguousarray(Wq[rows, :].T, dtype=np.float32),
            "wkT": np.ascontiguousarray(Wk[rows, :].T, dtype=np.float32),
            "wvT": np.ascontiguousarray(Wv[rows, :].T, dtype=np.float32),
            "woT": np.ascontiguousarray(Wo[:, rows].T, dtype=np.float32),
            **consts,
        })
    return in_maps


def run(x, Wq, Wk, Wv, Wo, mask, trace=False):
    nc, plan, mode = _get_compiled(mask)
    in_maps = _make_in_maps(x, Wq, Wk, Wv, Wo, mask, mode)
    res = bass_utils.run_bass_kernel_spmd(
        nc, in_maps, core_ids=list(range(NCORES)), trace=trace)
    out = np.empty((B, S, D), np.float32)
    for b in range(B):
        acc = res.results[2 * b]["outT"] + res.results[2 * b + 1]["outT"]
        out[b] = acc.T
    return out, res


def kernel(x, Wq, Wk, Wv, Wo, mask):
    x = np.asarray(x, dtype=np.float32)
    Wq = np.asarray(Wq, dtype=np.float32)
    Wk = np.asarray(Wk, dtype=np.float32)
    Wv = np.asarray(Wv, dtype=np.float32)
    Wo = np.asarray(Wo, dtype=np.float32)
    out, _ = run(x, Wq, Wk, Wv, Wo, mask)
    return out


# revision 17
# speedup vs baseline: 1.2539x; 1.2539x over previous
"""Multi-head attention (RoPE, causal) Trainium2 Bass kernel, 8 NeuronCores.

Problem: x[4,2048,1024] -> MHA(16 heads, head_dim 64, RoPE, causal mask) -> [4,2048,1024]

Sharding (pure data/tensor parallel, no collectives):
  core c -> (batch b = c//2, head-group g = c%2); each head-group = 8 heads = 512 dims.
  Each core computes q/k/v projections for its (batch, head-group), RoPE, attention,
  and a partial output projection (columns of Wo for its head group).
  Host sums the two partial outputs per batch (512-dim contraction split).

Kernel layout tricks:
  - Projections computed in transposed [out_dim, seq] layout (QT/KT) so that
    QK^T blocks come out as S^T [k, q]: softmax reductions along the partition
    dim are avoided entirely via UNSAFE softmax (no row-max; inputs are bounded
    N(0,1)-ish data, logits stay << 88) and the row-sum is folded into the PV
    matmul by augmenting V with a ones column.  No on-chip transposes anywhere.
  - V is computed in natural [seq, dim] layout (lhsT = xT chunks) for PV.
  - RoPE via a signed-permutation matrix on the TensorEngine (rot = Psig @ pre)
    plus 3 VectorEngine elementwise ops per chunk; the final add writes bf16
    Q/K tiles directly.
  - Projections / out-proj in float32r (full-rate fp32, moving dim >= 256);
    attention QK'/PV in bf16 (fast weight load, cheap LDWEIGHTS).
  - Causal masking: lower blocks computed unmasked, diagonal-strip blocks get a
    0/1 bf16 mask multiply; upper blocks skipped entirely.
  - Softmax normalization: 1/l batched on 4-head tiles (partitions 0/32/64/96),
    PE-broadcast of 1/l over each head's 64 rows, fused multiply into aT.
"""

import numpy as np

import concourse.bass as bass
import concourse.tile as tile
from concourse import bacc, mybir
from concourse import bass_utils

B, S, D, H, DH = 4, 2048, 1024, 16, 64
NCORES = 8
HG = 2              # head groups (tensor parallel)
HPG = H // HG       # heads per group = 8
OG = HPG * DH       # group output dims = 512
SCALE = DH ** -0.5
P = 128
QSB = 512           # q super-block width
NQSB = S // QSB     # 4
KB = 128            # k block
NKB = S // KB       # 16
DC = D // P         # 8 d-chunks
JC = OG // P        # 4 j-chunks (out-proj contraction)

F32 = mybir.dt.float32
F32R = mybir.dt.float32r
BF16 = mybir.dt.bfloat16

_COMPILED = {}


# ---------------------------------------------------------------- host tables

def _rope_tables():
    inv_freq = 1.0 / (10000.0 ** (np.arange(0, DH, 2, dtype=np.float32) / DH))
    t = np.arange(S, dtype=np.float32)
    freqs = np.outer(t, inv_freq).astype(np.float32)      # [S, 32]
    emb = np.concatenate([freqs, freqs], -1)              # [S, 64]
    return np.cos(emb), np.sin(emb)


def _host_consts():
    cos, sin = _rope_tables()                             # [S, 64]
    cosT2 = np.ascontiguousarray(
        np.concatenate([cos.T, cos.T], axis=0), dtype=np.float32)   # [128, S]
    sinT2 = np.ascontiguousarray(
        np.concatenate([sin.T, sin.T], axis=0), dtype=np.float32)
    # signed permutation: rot(x)[i] = -x[i+32] (j<32) else x[i-32], per 64-row head
    psig = np.zeros((P, P), np.float32)
    for i in range(P):
        j = i % DH
        base = (i // DH) * DH
        if j < 32:
            psig[i, base + j + 32] = -1.0
        else:
            psig[i, base + j - 32] = 1.0
    psigT = np.ascontiguousarray(psig.T)
    return cosT2, sinT2, psigT


def _mask_plan(mask):
    """Classify the [S, S] mask into a per-qsb block plan.

    plan[qsb] = list of (kb, msel); msel is None (no mask), ("const", r) for
    the 4 shared causal diagonal tiles, or ("dram", qsb, kb) for generic
    per-block mask tiles.
    """
    m = np.asarray(mask).reshape(S, S) != 0        # [q, k] True = attend
    causal = np.array_equal(m, np.tril(np.ones((S, S), bool)))
    if causal:
        plan = []
        for qsb in range(NQSB):
            row = []
            for kb in range(4 * qsb + 4):
                r = kb - 4 * qsb
                row.append((kb, None if r < 0 else ("const", r)))
            plan.append(row)
        return plan, "causal"
    if m.all():
        return [[(kb, None) for kb in range(NKB)] for _ in range(NQSB)], "full"
    plan = []
    for qsb in range(NQSB):
        row = []
        for kb in range(NKB):
            blk = m[qsb * QSB:(qsb + 1) * QSB, kb * KB:(kb + 1) * KB]  # [q, k]
            if not blk.any():
                continue          # fully masked block contributes nothing
            row.append((kb, None if blk.all() else ("dram", qsb, kb)))
        plan.append(row)
    return plan, "generic"


# ------------------------------------------------------------------- builder

def _build(plan, mode):
    nc = bacc.Bacc("TRN2", target_bir_lowering=False, debug=False, num_devices=1)
    AF = mybir.ActivationFunctionType
    OP = mybir.AluOpType

    xT_d = nc.dram_tensor("xT", [D, S], F32R, kind="ExternalInput").ap()
    wqT_d = nc.dram_tensor("wqT", [D, OG], F32R, kind="ExternalInput").ap()
    wkT_d = nc.dram_tensor("wkT", [D, OG], F32R, kind="ExternalInput").ap()
    wvT_d = nc.dram_tensor("wvT", [D, OG], F32R, kind="ExternalInput").ap()
    woT_d = nc.dram_tensor("woT", [OG, D], F32R, kind="ExternalInput").ap()
    cos_d = nc.dram_tensor("cosT", [P, S], F32, kind="ExternalInput").ap()
    sin_d = nc.dram_tensor("sinT", [P, S], F32, kind="ExternalInput").ap()
    psg_d = nc.dram_tensor("psgT", [P, P], F32R, kind="ExternalInput").ap()
    if mode == "causal":
        m01_d = nc.dram_tensor("m01", [4, P, QSB], F32, kind="ExternalInput").ap()
    elif mode == "generic":
        m01_d = nc.dram_tensor("m01", [NQSB, NKB, P, QSB], F32,
                               kind="ExternalInput").ap()
    else:
        m01_d = None
    one64_d = nc.dram_tensor("one64", [1, DH], F32R, kind="ExternalInput").ap()
    outT_d = nc.dram_tensor("outT", [D, S], F32, kind="ExternalOutput").ap()

    with tile.TileContext(nc) as tc:
        from contextlib import ExitStack
        with ExitStack() as ctx:
            persist = ctx.enter_context(tc.tile_pool(name="persist", bufs=1))
            wstream = ctx.enter_context(tc.tile_pool(name="wstream", bufs=2))
            work = ctx.enter_context(tc.tile_pool(name="work", bufs=2))
            prepool = ctx.enter_context(tc.tile_pool(name="prepool", bufs=3))
            ptpool = ctx.enter_context(tc.tile_pool(name="ptpool", bufs=4))
            ppool = ctx.enter_context(
                tc.tile_pool(name="ppool", bufs=4, space="PSUM"))
            pvp = ctx.enter_context(
                tc.tile_pool(name="pvp", bufs=3, space="PSUM"))

            # bf16 post-rope Q/K and bf16 V (with ones column) live all-kernel
            QTb = [persist.tile([P, S], BF16, tag=f"qt{t}", name=f"qtb{t}")
                   for t in range(4)]
            KTb = [persist.tile([P, S], BF16, tag=f"kt{t}", name=f"ktb{t}")
                   for t in range(4)]
            V = persist.tile([P, NKB, HPG, DH + 1], BF16, tag="v")
            nc.vector.memset(V[:, :, :, DH:DH + 1], 1.0)

            # ---------------- phase 1: projections + rope ----------------
            ph1 = tc.tile_pool(name="ph1", bufs=1)
            p1 = ph1.__enter__()

            xT = p1.tile([P, DC, S], F32R)
            for sc in range(4):
                nc.sync.dma_start(
                    xT[:, :, sc * QSB:(sc + 1) * QSB],
                    xT_d.rearrange("(c p) s -> p c s", p=P)
                        [:, :, sc * QSB:(sc + 1) * QSB])
            cos_sb = p1.tile([P, S], F32)
            sin_sb = p1.tile([P, S], F32)
            nc.sync.dma_start(cos_sb[:], cos_d)
            nc.sync.dma_start(sin_sb[:], sin_d)
            psg_sb = p1.tile([P, P], F32R)
            nc.sync.dma_start(psg_sb[:], psg_d)

            # Q/K projections (transposed [o, s] layout) fused with RoPE:
            # psum -> pre (f32r) -> {rot via PE, m/r via DVE} -> bf16 QTb/KTb
            for w_d, dst in ((wqT_d, QTb), (wkT_d, KTb)):
                for oc in range(4):
                    w_oc = wstream.tile([P, DC, P], F32R, tag="wqk")
                    nc.sync.dma_start(
                        w_oc[:],
                        w_d.rearrange("(c p) o -> p c o", p=P)
                           [:, :, oc * P:(oc + 1) * P])
                    for sc in range(4):
                        sl = slice(sc * QSB, (sc + 1) * QSB)
                        ps = ppool.tile([P, QSB], F32, tag="ps", name="ps")
                        for dc in range(DC):
                            nc.tensor.matmul(
                                ps[:],
                                w_oc[:, dc, :],
                                xT[:, dc, sl],
                                start=(dc == 0), stop=(dc == DC - 1))
                        pre = prepool.tile([P, QSB], F32R, tag="pre")
                        nc.scalar.copy(pre[:], ps[:])
                        rot = ppool.tile([P, QSB], F32, tag="ps", name="rot")
                        nc.tensor.matmul(
                            rot[:], psg_sb[:], pre[:], start=True, stop=True)
                        m = work.tile([P, QSB], F32, tag="ropem")
                        nc.vector.tensor_tensor(
                            m[:], pre[:], cos_sb[:, sl], OP.mult)
                        r = work.tile([P, QSB], F32, tag="roper")
                        nc.vector.tensor_tensor(
                            r[:], rot[:], sin_sb[:, sl], OP.mult)
                        nc.vector.tensor_tensor(
                            dst[oc][:, sl], m[:], r[:], OP.add)

            # V projection -> natural [s, o] layout, bf16 with ones column
            wv = p1.tile([P, DC, OG], F32R)
            nc.sync.dma_start(wv[:], wvT_d.rearrange("(c p) o -> p c o", p=P))
            for sb in range(NKB):
                ps = ppool.tile([P, QSB], F32, tag="ps", name="ps")
                for dc in range(DC):
                    nc.tensor.matmul(
                        ps[:],
                        xT[:, dc, sb * P:(sb + 1) * P],
                        wv[:, dc, :],
                        start=(dc == 0), stop=(dc == DC - 1))
                nc.scalar.copy(
                    V[:, sb, :, 0:DH],
                    ps[:].rearrange("p (h j) -> p h j", j=DH))

            ph1.__exit__(None, None, None)

            # ---------------- phase 2: attention ----------------
            ph2 = tc.tile_pool(name="ph2", bufs=1)
            p2 = ph2.__enter__()

            aT = [p2.tile([P, S], F32R, tag=f"at{t}", name=f"at{t}")
                  for t in range(4)]
            ones64 = p2.tile([1, DH], F32R, tag="ones64")
            nc.sync.dma_start(ones64[:], one64_d)
            if mode == "causal":
                mk = p2.tile([P, 4, QSB], BF16, tag="m01")
                for r in range(4):
                    mkf = work.tile([P, QSB], F32, tag="ropem", name="mkf")
                    nc.sync.dma_start(mkf[:], m01_d[r])
                    nc.vector.tensor_copy(mk[:, r, :], mkf[:])

            for h in range(HPG):
                tq = h // 2
                ph = (h % 2) * DH
                for qsb in range(NQSB):
                    qsl = slice(qsb * QSB, (qsb + 1) * QSB)
                    q_ap = QTb[tq][ph:ph + DH, qsl]
                    blocks = plan[qsb]
                    pv = pvp.tile([DH + 1, QSB], F32, tag="pv", name="pv")
                    for i, (kb, msel) in enumerate(blocks):
                        st = ppool.tile([P, QSB], F32, tag="ps", name="st")
                        nc.tensor.matmul(
                            st[:],
                            KTb[tq][ph:ph + DH, kb * KB:(kb + 1) * KB],
                            q_ap,
                            start=True, stop=True)
                        pt = ptpool.tile([P, QSB], BF16, tag="pt")
                        nc.scalar.activation(pt[:], st[:], AF.Exp, scale=SCALE)
                        if msel is not None:
                            if msel[0] == "const":
                                nc.vector.tensor_tensor(
                                    pt[:], pt[:], mk[:, msel[1], :], OP.mult)
                            else:
                                mg = work.tile([P, QSB], F32, tag="ropem")
                                nc.sync.dma_start(
                                    mg[:], m01_d[msel[1], msel[2]])
                                mgb = ptpool.tile([P, QSB], BF16, tag="pt")
                                nc.vector.tensor_copy(mgb[:], mg[:])
                                nc.vector.tensor_tensor(
                                    pt[:], pt[:], mgb[:], OP.mult)
                        nc.tensor.matmul(
                            pv[:],
                            V[:, kb, h, :],
                            pt[:],
                            start=(i == 0), stop=(i == len(blocks) - 1))
                    # normalize: broadcast l via PE, 1/ via fast approx,
                    # fused psum*sbuf multiply writes aT directly
                    lrow = work.tile([1, QSB], F32R, tag="lrow")
                    with nc.allow_low_precision(reason="f32r rounding of l"):
                        nc.vector.tensor_copy(lrow[:], pv[DH:DH + 1, :])
                    bc = ppool.tile([P, QSB], F32, tag="ps", name="bc")
                    nc.tensor.matmul(
                        bc[0:DH, :], ones64[:], lrow[:], start=True, stop=True)
                    binv = work.tile([DH, QSB], F32, tag="binv")
                    nc.vector.reciprocal_approx_fast(binv[:], bc[0:DH, :])
                    nc.vector.tensor_tensor(
                        aT[tq][ph:ph + DH, qsl], pv[0:DH, :],
                        binv[:], OP.mult)

            # output projection (partial over this head group's 512 dims)
            for oc in range(8):
                wo = wstream.tile([P, JC, P], F32R, tag="wo")
                nc.sync.dma_start(
                    wo[:],
                    woT_d.rearrange("(c p) o -> p c o", p=P)
                         [:, :, oc * P:(oc + 1) * P])
                for sc in range(4):
                    ssl = slice(sc * QSB, (sc + 1) * QSB)
                    ps = ppool.tile([P, QSB], F32, tag="ps", name="ps")
                    for jc in range(JC):
                        nc.tensor.matmul(
                            ps[:], wo[:, jc, :],
                            aT[jc][:, ssl],
                            start=(jc == 0), stop=(jc == JC - 1))
                    stg = work.tile([P, QSB], F32, tag="ropem", name="stg")
                    nc.vector.tensor_copy(stg[:], ps[:])
                    nc.sync.dma_start(outT_d[oc * P:(oc + 1) * P, ssl], stg[:])

            ph2.__exit__(None, None, None)

    nc.compile()
    return nc


def _plan_key(plan, mode):
    return (mode, tuple(tuple(row) for row in plan))


def _get_compiled(mask):
    plan, mode = _mask_plan(mask)
    key = _plan_key(plan, mode)
    if key not in _COMPILED:
        _COMPILED[key] = (_build(plan, mode), plan, mode)
    return _COMPILED[key]


# --------------------------------------------------------------- host driver

def _make_in_maps(x, Wq, Wk, Wv, Wo, mask, mode):
    cosT2, sinT2, psigT = _host_consts()
    consts = {"cosT": cosT2, "sinT": sinT2, "psgT": psigT,
              "one64": np.ones((1, DH), np.float32)}
    if mode == "causal":
        m01 = np.zeros((4, P, QSB), np.float32)
        for r in range(4):
            for k in range(P):
                q0 = KB * r + k
                if q0 < QSB:
                    m01[r, k, q0:] = 1.0
        consts["m01"] = m01
    elif mode == "generic":
        m = (np.asarray(mask).reshape(S, S) != 0)
        m01 = np.zeros((NQSB, NKB, P, QSB), np.float32)
        for qsb in range(NQSB):
            for kb in range(NKB):
                blk = m[qsb * QSB:(qsb + 1) * QSB, kb * KB:(kb + 1) * KB]
                m01[qsb, kb] = blk.T.astype(np.float32)
        consts["m01"] = m01

    in_maps = []
    for c in range(NCORES):
        b, g = c // HG, c % HG
        rows = slice(OG * g, OG * (g + 1))
        in_maps.append({
            "xT": np.ascontiguousarray(x[b].T, dtype=np.float32),
            "wqT": np.ascontiguousarray(Wq[rows, :].T, dtype=np.float32),
            "wkT": np.ascontiguousarray(Wk[rows, :].T, dtype=np.float32),
            "wvT": np.ascontiguousarray(Wv[rows, :].T, dtype=np.float32),
            "woT": np.ascontiguousarray(Wo[:, rows].T, dtype=np.float32),
            **consts,
        })
    return in_maps


def run(x, Wq, Wk, Wv, Wo, mask, trace=False):
    nc, plan, mode = _get_compiled(mask)
    in_maps = _make_in_maps(x, Wq, Wk, Wv, Wo, mask, mode)
    res = bass_utils.run_bass_kernel_spmd(
        nc, in_maps, core_ids=list(range(NCORES)), trace=trace)
    out = np.empty((B, S, D), np.float32)
    for b in range(B):
        acc = res.results[2 * b]["outT"] + res.results[2 * b + 1]["outT"]
        out[b] = acc.T
    return out, res


def kernel(x, Wq, Wk, Wv, Wo, mask):
    x = np.asarray(x, dtype=np.float32)
    Wq = np.asarray(Wq, dtype=np.float32)
    Wk = np.asarray(Wk, dtype=np.float32)
    Wv = np.asarray(Wv, dtype=np.float32)
    Wo = np.asarray(Wo, dtype=np.float32)
    out, _ = run(x, Wq, Wk, Wv, Wo, mask)
    return out


# revision 19
# speedup vs baseline: 1.3636x; 1.0875x over previous
"""Multi-head attention (RoPE, causal) Trainium2 Bass kernel, 8 NeuronCores.

Problem: x[4,2048,1024] -> MHA(16 heads, head_dim 64, RoPE, causal mask) -> [4,2048,1024]

Sharding (pure data/tensor parallel, no collectives):
  core c -> (batch b = c//2, head-group g = c%2); each head-group = 8 heads = 512 dims.
  Each core computes q/k/v projections for its (batch, head-group), RoPE, attention,
  and a partial output projection (columns of Wo for its head group).
  Host sums the two partial outputs per batch (512-dim contraction split).

Kernel layout tricks:
  - Projections computed in transposed [out_dim, seq] layout (QT/KT) so that
    QK^T blocks come out as S^T [k, q]: softmax reductions along the partition
    dim are avoided entirely via UNSAFE softmax (no row-max; inputs are bounded
    N(0,1)-ish data, logits stay << 88) and the row-sum is folded into the PV
    matmul by augmenting V with a ones column.  No on-chip transposes anywhere.
  - V is computed in natural [seq, dim] layout (lhsT = xT chunks) for PV.
  - RoPE via a signed-permutation matrix on the TensorEngine (rot = Psig @ pre)
    plus 3 VectorEngine elementwise ops per chunk; the final add writes bf16
    Q/K tiles directly.
  - Projections / out-proj in float32r (full-rate fp32, moving dim >= 256);
    attention QK'/PV in bf16 (fast weight load, cheap LDWEIGHTS).
  - Causal masking: lower blocks computed unmasked, diagonal-strip blocks get a
    0/1 bf16 mask multiply; upper blocks skipped entirely.
  - Softmax normalization: 1/l batched on 4-head tiles (partitions 0/32/64/96),
    PE-broadcast of 1/l over each head's 64 rows, fused multiply into aT.
"""

import numpy as np

import concourse.bass as bass
import concourse.tile as tile
from concourse import bacc, mybir
from concourse import bass_utils

B, S, D, H, DH = 4, 2048, 1024, 16, 64
NCORES = 8
HG = 2              # head groups (tensor parallel)
HPG = H // HG       # heads per group = 8
OG = HPG * DH       # group output dims = 512
SCALE = DH ** -0.5
P = 128
QSB = 512           # q super-block width
NQSB = S // QSB     # 4
KB = 128            # k block
NKB = S // KB       # 16
DC = D // P         # 8 d-chunks
JC = OG // P        # 4 j-chunks (out-proj contraction)

F32 = mybir.dt.float32
F32R = mybir.dt.float32r
BF16 = mybir.dt.bfloat16

_COMPILED = {}


# ---------------------------------------------------------------- host tables

def _rope_tables():
    inv_freq = 1.0 / (10000.0 ** (np.arange(0, DH, 2, dtype=np.float32) / DH))
    t = np.arange(S, dtype=np.float32)
    freqs = np.outer(t, inv_freq).astype(np.float32)      # [S, 32]
    emb = np.concatenate([freqs, freqs], -1)              # [S, 64]
    return np.cos(emb), np.sin(emb)


def _host_consts():
    cos, sin = _rope_tables()                             # [S, 64]
    cosT2 = np.ascontiguousarray(
        np.concatenate([cos.T, cos.T], axis=0), dtype=np.float32)   # [128, S]
    sinT2 = np.ascontiguousarray(
        np.concatenate([sin.T, sin.T], axis=0), dtype=np.float32)
    # signed permutation: rot(x)[i] = -x[i+32] (j<32) else x[i-32], per 64-row head
    psig = np.zeros((P, P), np.float32)
    for i in range(P):
        j = i % DH
        base = (i // DH) * DH
        if j < 32:
            psig[i, base + j + 32] = -1.0
        else:
            psig[i, base + j - 32] = 1.0
    psigT = np.ascontiguousarray(psig.T)
    return cosT2, sinT2, psigT


def _mask_plan(mask):
    """Classify the [S, S] mask into a per-qsb block plan.

    plan[qsb] = list of (kb, msel); msel is None (no mask), ("const", r) for
    the 4 shared causal diagonal tiles, or ("dram", qsb, kb) for generic
    per-block mask tiles.
    """
    m = np.asarray(mask).reshape(S, S) != 0        # [q, k] True = attend
    causal = np.array_equal(m, np.tril(np.ones((S, S), bool)))
    if causal:
        plan = []
        for qsb in range(NQSB):
            row = []
            for kb in range(4 * qsb + 4):
                r = kb - 4 * qsb
                row.append((kb, None if r < 0 else ("const", r)))
            plan.append(row)
        return plan, "causal"
    if m.all():
        return [[(kb, None) for kb in range(NKB)] for _ in range(NQSB)], "full"
    plan = []
    for qsb in range(NQSB):
        row = []
        for kb in range(NKB):
            blk = m[qsb * QSB:(qsb + 1) * QSB, kb * KB:(kb + 1) * KB]  # [q, k]
            if not blk.any():
                continue          # fully masked block contributes nothing
            row.append((kb, None if blk.all() else ("dram", qsb, kb)))
        plan.append(row)
    return plan, "generic"


# ------------------------------------------------------------------- builder

def _build(plan, mode):
    nc = bacc.Bacc("TRN2", target_bir_lowering=False, debug=False, num_devices=1)
    AF = mybir.ActivationFunctionType
    OP = mybir.AluOpType

    xT_d = nc.dram_tensor("xT", [D, S], F32R, kind="ExternalInput").ap()
    wqT_d = nc.dram_tensor("wqT", [D, OG], F32R, kind="ExternalInput").ap()
    wkT_d = nc.dram_tensor("wkT", [D, OG], F32R, kind="ExternalInput").ap()
    wvT_d = nc.dram_tensor("wvT", [D, OG], F32R, kind="ExternalInput").ap()
    woT_d = nc.dram_tensor("woT", [OG, D], F32R, kind="ExternalInput").ap()
    cos_d = nc.dram_tensor("cosT", [P, S], F32, kind="ExternalInput").ap()
    sin_d = nc.dram_tensor("sinT", [P, S], F32, kind="ExternalInput").ap()
    psg_d = nc.dram_tensor("psgT", [P, P], F32R, kind="ExternalInput").ap()
    if mode == "causal":
        m01_d = nc.dram_tensor("m01", [4, P, QSB], F32, kind="ExternalInput").ap()
    elif mode == "generic":
        m01_d = nc.dram_tensor("m01", [NQSB, NKB, P, QSB], F32,
                               kind="ExternalInput").ap()
    else:
        m01_d = None
    one64_d = nc.dram_tensor("one64", [1, DH], F32R, kind="ExternalInput").ap()
    outT_d = nc.dram_tensor("outT", [D, S], F32, kind="ExternalOutput").ap()

    with tile.TileContext(nc) as tc:
        from contextlib import ExitStack
        with ExitStack() as ctx:
            persist = ctx.enter_context(tc.tile_pool(name="persist", bufs=1))
            wstream = ctx.enter_context(tc.tile_pool(name="wstream", bufs=2))
            work = ctx.enter_context(tc.tile_pool(name="work", bufs=2))
            prepool = ctx.enter_context(tc.tile_pool(name="prepool", bufs=3))
            ptpool = ctx.enter_context(tc.tile_pool(name="ptpool", bufs=4))
            ppool = ctx.enter_context(
                tc.tile_pool(name="ppool", bufs=3, space="PSUM"))
            pvp = ctx.enter_context(
                tc.tile_pool(name="pvp", bufs=2, space="PSUM"))

            # bf16 post-rope Q/K and bf16 V (with ones column) live all-kernel
            QTb = [persist.tile([P, S], BF16, tag=f"qt{t}", name=f"qtb{t}")
                   for t in range(4)]
            KTb = [persist.tile([P, S], BF16, tag=f"kt{t}", name=f"ktb{t}")
                   for t in range(4)]
            V = persist.tile([P, NKB, HPG, DH + 1], BF16, tag="v")
            nc.vector.memset(V[:, :, :, DH:DH + 1], 1.0)

            # ---------------- phase 1: projections + rope ----------------
            ph1 = tc.tile_pool(name="ph1", bufs=1)
            p1 = ph1.__enter__()

            xT = p1.tile([P, DC, S], F32R)
            for sc in range(4):
                nc.sync.dma_start(
                    xT[:, :, sc * QSB:(sc + 1) * QSB],
                    xT_d.rearrange("(c p) s -> p c s", p=P)
                        [:, :, sc * QSB:(sc + 1) * QSB])
            cos_sb = p1.tile([P, S], F32)
            sin_sb = p1.tile([P, S], F32)
            nc.sync.dma_start(cos_sb[:], cos_d)
            nc.sync.dma_start(sin_sb[:], sin_d)
            psg_sb = p1.tile([P, P], F32R)
            nc.sync.dma_start(psg_sb[:], psg_d)

            # Q/K projections (transposed [o, s] layout) fused with RoPE:
            # psum -> pre (f32r) -> {rot via PE, m/r via DVE} -> bf16 QTb/KTb
            for w_d, dst in ((wqT_d, QTb), (wkT_d, KTb)):
                for oc in range(4):
                    w_oc = wstream.tile([P, DC, P], F32R, tag="wqk")
                    nc.sync.dma_start(
                        w_oc[:],
                        w_d.rearrange("(c p) o -> p c o", p=P)
                           [:, :, oc * P:(oc + 1) * P])
                    for sc in range(4):
                        sl = slice(sc * QSB, (sc + 1) * QSB)
                        ps = ppool.tile([P, QSB], F32, tag="ps", name="ps")
                        for dc in range(DC):
                            nc.tensor.matmul(
                                ps[:],
                                w_oc[:, dc, :],
                                xT[:, dc, sl],
                                start=(dc == 0), stop=(dc == DC - 1))
                        pre = prepool.tile([P, QSB], F32R, tag="pre")
                        nc.scalar.copy(pre[:], ps[:])
                        rot = ppool.tile([P, QSB], F32, tag="ps", name="rot")
                        nc.tensor.matmul(
                            rot[:], psg_sb[:], pre[:], start=True, stop=True)
                        m = work.tile([P, QSB], F32, tag="ropem")
                        nc.vector.tensor_tensor(
                            m[:], pre[:], cos_sb[:, sl], OP.mult)
                        r = work.tile([P, QSB], F32, tag="roper")
                        nc.vector.tensor_tensor(
                            r[:], rot[:], sin_sb[:, sl], OP.mult)
                        nc.vector.tensor_tensor(
                            dst[oc][:, sl], m[:], r[:], OP.add)

            # V projection -> natural [s, o] layout, bf16 with ones column
            wv = p1.tile([P, DC, OG], F32R)
            nc.sync.dma_start(wv[:], wvT_d.rearrange("(c p) o -> p c o", p=P))
            for sb in range(NKB):
                ps = ppool.tile([P, QSB], F32, tag="ps", name="ps")
                for dc in range(DC):
                    nc.tensor.matmul(
                        ps[:],
                        xT[:, dc, sb * P:(sb + 1) * P],
                        wv[:, dc, :],
                        start=(dc == 0), stop=(dc == DC - 1))
                nc.scalar.copy(
                    V[:, sb, :, 0:DH],
                    ps[:].rearrange("p (h j) -> p h j", j=DH))

            ph1.__exit__(None, None, None)

            # ---------------- phase 2: attention ----------------
            ph2 = tc.tile_pool(name="ph2", bufs=1)
            p2 = ph2.__enter__()

            aT = [p2.tile([P, S], F32R, tag=f"at{t}", name=f"at{t}")
                  for t in range(4)]
            ones64 = p2.tile([1, DH], F32R, tag="ones64")
            nc.sync.dma_start(ones64[:], one64_d)
            if mode == "causal":
                mk = p2.tile([P, 4, QSB], BF16, tag="m01")
                for r in range(4):
                    mkf = work.tile([P, QSB], F32, tag="ropem", name="mkf")
                    nc.sync.dma_start(mkf[:], m01_d[r])
                    nc.vector.tensor_copy(mk[:, r, :], mkf[:])

            for h in range(HPG):
                tq = h // 2
                ph = (h % 2) * DH
                for qsb in range(NQSB):
                    qsl = slice(qsb * QSB, (qsb + 1) * QSB)
                    q_ap = QTb[tq][ph:ph + DH, qsl]
                    blocks = plan[qsb]
                    pv = pvp.tile([DH + 1, QSB], F32, tag="pv", name="pv")
                    bi = 0
                    for p0 in range(0, len(blocks), 2):
                        pair = blocks[p0:p0 + 2]
                        w = len(pair)
                        # two k-blocks share one [128, 1024] psum tile (2 banks)
                        # so a single wide Exp covers both
                        st2 = ppool.tile([P, 2, QSB], F32, tag="ps", name="st2")
                        for j, (kb, msel) in enumerate(pair):
                            nc.tensor.matmul(
                                st2[:, j, :],
                                KTb[tq][ph:ph + DH, kb * KB:(kb + 1) * KB],
                                q_ap,
                                start=True, stop=True)
                        pt2 = ptpool.tile([P, 2, QSB], BF16, tag="pt")
                        nc.scalar.activation(
                            pt2[:, 0:w, :], st2[:, 0:w, :], AF.Exp, scale=SCALE)
                        for j, (kb, msel) in enumerate(pair):
                            if msel is not None:
                                if msel[0] == "const":
                                    nc.vector.tensor_tensor(
                                        pt2[:, j, :], pt2[:, j, :],
                                        mk[:, msel[1], :], OP.mult)
                                else:
                                    mg = work.tile([P, QSB], F32, tag="ropem")
                                    nc.sync.dma_start(
                                        mg[:], m01_d[msel[1], msel[2]])
                                    mgb = ptpool.tile(
                                        [P, 2, QSB], BF16, tag="pt", name="mgb")
                                    nc.vector.tensor_copy(mgb[:, 0, :], mg[:])
                                    nc.vector.tensor_tensor(
                                        pt2[:, j, :], pt2[:, j, :],
                                        mgb[:, 0, :], OP.mult)
                            nc.tensor.matmul(
                                pv[:],
                                V[:, kb, h, :],
                                pt2[:, j, :],
                                start=(bi == 0), stop=(bi == len(blocks) - 1))
                            bi += 1
                    # normalize: broadcast l via PE, 1/ via fast approx,
                    # fused psum*sbuf multiply writes aT directly
                    lrow = work.tile([1, QSB], F32R, tag="lrow")
                    with nc.allow_low_precision(reason="f32r rounding of l"):
                        nc.vector.tensor_copy(lrow[:], pv[DH:DH + 1, :])
                    bc = ppool.tile([P, QSB], F32, tag="ps", name="bc")
                    nc.tensor.matmul(
                        bc[0:DH, :], ones64[:], lrow[:], start=True, stop=True)
                    binv = work.tile([DH, QSB], F32, tag="binv")
                    nc.vector.reciprocal_approx_fast(binv[:], bc[0:DH, :])
                    nc.vector.tensor_tensor(
                        aT[tq][ph:ph + DH, qsl], pv[0:DH, :],
                        binv[:], OP.mult)

            # output projection (partial over this head group's 512 dims)
            for oc in range(8):
                wo = wstream.tile([P, JC, P], F32R, tag="wo")
                nc.sync.dma_start(
                    wo[:],
                    woT_d.rearrange("(c p) o -> p c o", p=P)
                         [:, :, oc * P:(oc + 1) * P])
                for sc in range(4):
                    ssl = slice(sc * QSB, (sc + 1) * QSB)
                    ps = ppool.tile([P, QSB], F32, tag="ps", name="ps")
                    for jc in range(JC):
                        nc.tensor.matmul(
                            ps[:], wo[:, jc, :],
                            aT[jc][:, ssl],
                            start=(jc == 0), stop=(jc == JC - 1))
                    stg = work.tile([P, QSB], F32, tag="ropem", name="stg")
                    nc.vector.tensor_copy(stg[:], ps[:])
                    nc.sync.dma_start(outT_d[oc * P:(oc + 1) * P, ssl], stg[:])

            ph2.__exit__(None, None, None)

    nc.compile()
    return nc


def _plan_key(plan, mode):
    return (mode, tuple(tuple(row) for row in plan))


def _get_compiled(mask):
    plan, mode = _mask_plan(mask)
    key = _plan_key(plan, mode)
    if key not in _COMPILED:
        _COMPILED[key] = (_build(plan, mode), plan, mode)
    return _COMPILED[key]


# --------------------------------------------------------------- host driver

def _make_in_maps(x, Wq, Wk, Wv, Wo, mask, mode):
    cosT2, sinT2, psigT = _host_consts()
    consts = {"cosT": cosT2, "sinT": sinT2, "psgT": psigT,
              "one64": np.ones((1, DH), np.float32)}
    if mode == "causal":
        m01 = np.zeros((4, P, QSB), np.float32)
        for r in range(4):
            for k in range(P):
                q0 = KB * r + k
                if q0 < QSB:
                    m01[r, k, q0:] = 1.0
        consts["m01"] = m01
    elif mode == "generic":
        m = (np.asarray(mask).reshape(S, S) != 0)
        m01 = np.zeros((NQSB, NKB, P, QSB), np.float32)
        for qsb in range(NQSB):
            for kb in range(NKB):
                blk = m[qsb * QSB:(qsb + 1) * QSB, kb * KB:(kb + 1) * KB]
                m01[qsb, kb] = blk.T.astype(np.float32)
        consts["m01"] = m01

    in_maps = []
    for c in range(NCORES):
        b, g = c // HG, c % HG
        rows = slice(OG * g, OG * (g + 1))
        in_maps.append({
            "xT": np.ascontiguousarray(x[b].T, dtype=np.float32),
            "wqT": np.ascontiguousarray(Wq[rows, :].T, dtype=np.float32),
            "wkT": np.ascontiguousarray(Wk[rows, :].T, dtype=np.float32),
            "wvT": np.ascontiguousarray(Wv[rows, :].T, dtype=np.float32),
            "woT": np.ascontiguousarray(Wo[:, rows].T, dtype=np.float32),
            **consts,
        })
    return in_maps


def run(x, Wq, Wk, Wv, Wo, mask, trace=False):
    nc, plan, mode = _get_compiled(mask)
    in_maps = _make_in_maps(x, Wq, Wk, Wv, Wo, mask, mode)
    res = bass_utils.run_bass_kernel_spmd(
        nc, in_maps, core_ids=list(range(NCORES)), trace=trace)
    out = np.empty((B, S, D), np.float32)
    for b in range(B):
        acc = res.results[2 * b]["outT"] + res.results[2 * b + 1]["outT"]
        out[b] = acc.T
    return out, res


def kernel(x, Wq, Wk, Wv, Wo, mask):
    x = np.asarray(x, dtype=np.float32)
    Wq = np.asarray(Wq, dtype=np.float32)
    Wk = np.asarray(Wk, dtype=np.float32)
    Wv = np.asarray(Wv, dtype=np.float32)
    Wo = np.asarray(Wo, dtype=np.float32)
    out, _ = run(x, Wq, Wk, Wv, Wo, mask)
    return out
